# revision 29
# baseline (speedup 1.0000x reference)
"""CPDBlock (rank-decomposed conv block) Trainium2 kernel.

Reference computation (per image):
  y1 = (sum_r w_head[r]) @ x            # 1x1 conv, 256->256
  y2 = conv_(3,1)(y1, w_body)           # 256->64, pad (1,0) in H
  y3 = conv_(1,3)(y2, w_tail) + b_tail  # 64->256, pad (0,1) in W

Algebraic fusion: head folds into body since both are linear:
  y2[r,h,w] = sum_kh (w_body[:, :, kh] @ w_sum) @ x[:, h+kh-1, w]
so the kernel only runs two conv stages:
  fused:  Wc[kh] = w_body[kh] @ w_head.sum(0)  (3x [64,256], host-side)
  tail:   w_tail as-is, bias applied on the PSUM->SBUF evacuation.

Sharding: data-parallel over batch, 16 images / 8 cores = 2 images/core.
Everything on the PE runs fp16 (x, fused weights, y2, tail weights) with
fp32 PSUM accumulation; rel err ~5e-4.

HBM layouts are partition-major ([128, ...]) so DMA descriptor lines are
3.6KB contiguous instead of 224B; the host transposes x/o accordingly.

Per core, per image, H in chunks of HC rows; groups of NR=4 output rows:
  x chunk  [128p=cin%128, 2=cin//128, HC+2 rows (halo), 112]  (SBUF, f16)
  y2d      [128p, HC rows, 114] f16: partitions 0-63 hold y2 row-padded
           (col0=0, cols1..112=data), partitions 64-127 the same shifted
           one col left (cols0..111=data, col112=0).  The tail's three
           shifted W-windows then come out as two K=128 matmuls.
  y3 stage [128p=cout%128, 2=cout//128, HC, 112] -> DMA out.

Every matmul is M=64 col-tiled (tile_position (0,0)/(0,64)) so the PE
array never changes tiling mode: fused pairs two row-groups across the
col halves; the tail splits its M=128 output across them.  Tail matmuls
for pair p are emitted after the fused matmuls of pair p+1 so the
PSUM->y2d evacuation latency stays off the tensor critical path.
"""
import os

import numpy as np

import concourse.bass as bass
import concourse.mybir as mybir
import concourse.tile as tile
from concourse import bacc
from concourse.bass_utils import run_bass_kernel_spmd

F32 = mybir.dt.float32
F16 = mybir.dt.float16

B, CIN, COUT, RANK, H, W = 16, 256, 256, 64, 112, 112
NCORES = 8
BL = B // NCORES          # images per core
KO = CIN // 128           # input-channel k-tiles
MO = COUT // 128          # output-channel m-tiles
HC = 56                   # rows per chunk
NCH = H // HC             # chunks per image
NR = 4                    # output rows per matmul group (N = NR*112 = 448)
NG = HC // NR             # groups per chunk

LAST_EXEC_NS = None
LAST_IN_MAPS = None


def _build():
    nc = bacc.Bacc("TRN2", target_bir_lowering=False, debug=False,
                   num_devices=NCORES)
    x_d = nc.dram_tensor("x", [128, BL, KO, H, W], F16, kind="ExternalInput")
    wf_d = nc.dram_tensor("wf", [128, 3, KO, RANK], F16, kind="ExternalInput")
    wt_d = nc.dram_tensor("wt", [128, MO, 2, 128], F16, kind="ExternalInput")
    bias_d = nc.dram_tensor("bias", [128, MO], F32, kind="ExternalInput")
    o_d = nc.dram_tensor("o", [128, BL, H, MO, W], F16, kind="ExternalOutput")

    with tile.TileContext(nc) as tc:
        with (
            tc.tile_pool(name="wpool", bufs=1) as wpool,
            tc.tile_pool(name="xpool", bufs=2) as xpool,
            tc.tile_pool(name="ypool", bufs=1) as ypool,
            tc.tile_pool(name="opool", bufs=2) as opool,
            tc.tile_pool(name="psf", bufs=4, space="PSUM") as psf,
            tc.tile_pool(name="pst", bufs=2, space="PSUM") as pst,
        ):
            wf = wpool.tile([128, 3, KO, RANK], F16)
            wt = wpool.tile([128, MO, 2, 128], F16)
            bias = wpool.tile([128, MO], F32)
            # Weights ride the Scalar engine's HWDGE ring so the Sync
            # ring's first issue is chunk 0's x piece (startup latency).
            nc.scalar.dma_start(wf[:], wf_d[:])
            nc.scalar.dma_start(wt[:], wt_d[:])
            nc.scalar.dma_start(bias[:], bias_d[:])

            # Two persistent y2d buffers, alternated per chunk.  Pad
            # columns (col0 of the top half, col112 of the bottom half)
            # are zeroed once and never rewritten.
            y2ds = [ypool.tile([128, HC, 114], F16, tag=f"y2d{i}",
                               name=f"y2d{i}")
                    for i in range(2)]
            # y2d pad zeroing rides GpSimd (idle early) so DVE is free
            # for the first evacuations the moment data arrives.
            for y2d in y2ds:
                nc.gpsimd.memset(y2d[:], 0.0)

            # HAM warm-up: throwaway col-tiled matmuls keep the PE busy
            # from ~5us so the clock gate opens (K=8/8) before the
            # first real matmul; their PSUM bank is overwritten
            # (start=True) later.
            warm = wpool.tile([128, 448], F16, name="warm")
            nc.vector.memset(warm[:], 0.0)
            wpsum = pst.tile([128, NR * W], F32, tag="pt0", name="pt0")
            for i in range(16):
                ct = 64 * (i % 2)
                nc.tensor.matmul(wpsum[ct:ct + 64, :], warm[:, 0:64],
                                 warm[:], start=True, stop=True,
                                 tile_position=(0, ct))

            it = 0
            pend = None       # deferred tail pair (gp, y2d, y3t)
            pend_out = None   # deferred second output slice (ov, h0, y3t)
            for b in range(BL):
                xv = x_d.ap()[:, b]           # [128, KO, H, W]
                ov = o_d.ap()[:, b]           # [128, H, MO, W]
                for ch in range(NCH):
                    h0 = ch * HC
                    xt = xpool.tile([128, KO, HC + 2, W], F16)
                    # xt slot i holds absolute image row h0 + i - 1;
                    # edge chunks leave the out-of-image slot unwritten
                    # and skip the matmul term that would read it.
                    # Rows h0-1, h0 are copied from the previous chunk's
                    # tile; each image row is DMA'd from HBM once.  The
                    # load is split into 8-row pieces so pair p's
                    # matmuls only wait for piece p.
                    if ch == 0:
                        lo = 1
                    else:
                        nc.gpsimd.tensor_copy(xt[:, :, 0:2, :],
                                              xt_prev[:, :, HC:HC + 2, :])
                        lo = 2
                    hi = HC + 2 if ch < NCH - 1 else HC + 1
                    bounds = [lo] + list(range(10, hi, 8)) + [hi]
                    bounds = sorted(set(b_ for b_ in bounds if lo <= b_ <= hi))
                    for s0, s1 in zip(bounds[:-1], bounds[1:]):
                        nc.sync.dma_start(
                            xt[:, :, s0:s1, :],
                            xv[:, :, h0 + s0 - 1:h0 + s1 - 1, :])
                    xt_prev = xt

                    y2d = y2ds[it % 2]
                    it += 1
                    # [HC, MO, W] free layout so the out-DMA is one
                    # contiguous 12.5KB run per partition per slice.
                    y3t = opool.tile([128, HC, MO, W], F16)

                    # ---- emitters -------------------------------------
                    def emit_fused_pair(gp, xt=xt, ch=ch):
                        """Fused-stage matmuls for groups gp, gp+1: group
                        gp lands in PSUM partitions 0:64 (PE col tiles
                        0/1), gp+1 in 64:128 (col tiles 2/3); the two
                        streams run concurrently."""
                        subs = [0, 1] if gp + 1 < NG else [0]
                        pfp = psf.tile([128, NR * W], F32)
                        for ko in range(KO):
                            for kh in (1, 0, 2):
                                for sub in subs:
                                    g = gp + sub
                                    r0 = g * NR
                                    p0 = 64 * sub
                                    out_ap = pfp[p0:p0 + 64, :]
                                    rhs = xt[:, ko, r0 + kh:r0 + kh + NR, :]
                                    if ch == 0 and g == 0 and kh == 0:
                                        # output row 0 has no row above
                                        out_ap = pfp[p0:p0 + 64, W:NR * W]
                                        rhs = xt[:, ko, 1:NR, :]
                                    elif (ch == NCH - 1 and g == NG - 1
                                          and kh == 2):
                                        # last row has no row below
                                        out_ap = pfp[p0:p0 + 64,
                                                     0:(NR - 1) * W]
                                        rhs = xt[:, ko, r0 + 2:r0 + 1 + NR, :]
                                    nc.tensor.matmul(
                                        out_ap,
                                        wf[:, kh, ko, :],
                                        rhs,
                                        start=(ko == 0 and kh == 1),
                                        stop=(ko == KO - 1 and kh == 2),
                                        tile_position=(0, p0),
                                    )
                        return pfp, subs

                    def emit_evac(gp, pfp, subs, y2d=y2d):
                        """PSUM -> y2d padded/shifted layout; one half on
                        ACT, one on DVE so neither engine is the
                        bottleneck."""
                        for sub in subs:
                            r0 = (gp + sub) * NR
                            pf = pfp[64 * sub:64 * sub + 64, :]
                            nc.scalar.copy(y2d[0:64, r0:r0 + NR, 1:113], pf)
                            nc.vector.tensor_copy(
                                y2d[64:128, r0:r0 + NR, 0:112], pf)

                    def emit_tail_pair(gp, y2d, y3t):
                        """Tail matmuls + biased evacuation for groups
                        gp, gp+1.  Each mo half is col-tiled into two
                        concurrent M=64 streams so the PE stays in
                        128x64 mode; bias rides the evacuation (ACT for
                        mo 0, DVE tensor_scalar for mo 1)."""
                        for sub in ([0, 1] if gp + 1 < NG else [0]):
                            g = gp + sub
                            r0 = g * NR
                            for mo in range(MO):
                                pt = pst.tile([128, NR * W], F32,
                                              tag=f"pt{mo}", name=f"pt{mo}")
                                for tap in range(2):
                                    rhs = y2d[:, r0:r0 + NR, tap:tap + 112]
                                    for ct in (0, 64):
                                        nc.tensor.matmul(
                                            pt[ct:ct + 64, :],
                                            wt[:, mo, tap, ct:ct + 64],
                                            rhs,
                                            start=(tap == 0),
                                            stop=(tap == 1),
                                            tile_position=(0, ct),
                                        )
                                out = y3t[:, r0:r0 + NR, mo, :]
                                if mo == 0:
                                    nc.scalar.add(out, pt[:],
                                                  bias[:, mo, None])
                                else:
                                    nc.vector.tensor_scalar_add(
                                        out, pt[:], bias[:, mo, None])

                    # ---- software-pipelined pair loop -----------------
                    # The tail-pair for pair p is emitted after pair
                    # p+1's fused matmuls (crossing chunk boundaries via
                    # `pend`) so the PSUM->y2d evacuation latency never
                    # heads-of-line-blocks the tensor queue.
                    for gp in range(0, NG, 2):
                        pfp, subs = emit_fused_pair(gp)
                        if pend is not None:
                            emit_tail_pair(pend[0], pend[1], pend[2])
                            pend = None
                            if pend_out is not None:
                                pov, ph0, py3t = pend_out
                                nc.gpsimd.dma_start(
                                    pov[:, ph0 + 28:ph0 + 56, :, :],
                                    py3t[:, 28:56, :, :])
                                pend_out = None
                        emit_evac(gp, pfp, subs)
                        pend = (gp, y2d, y3t)

                    # Output leaves on the GpSimd SWDGE ring so it never
                    # queues behind the x input stream.  The HBM layout
                    # interleaves MO inside rows so each partition is one
                    # 12.5KB contiguous run (128 descriptors per DMA
                    # keeps the Q7 descriptor-generation cost low).
                    # Rows 28:56 need the deferred tail pair; that slice
                    # is emitted right after it, next chunk.
                    last = (b == BL - 1 and ch == NCH - 1)
                    if not last:
                        nc.gpsimd.dma_start(ov[:, h0:h0 + 28, :, :],
                                            y3t[:, 0:28, :, :])
                        pend_out = (ov, h0, y3t)
                    else:
                        # Final chunk: x input is fully landed, so the
                        # Sync HWDGE ring is idle — drain the last
                        # output there in 14-row pieces to cut the
                        # post-compute DMA tail.
                        for s0 in range(0, 28, 14):
                            nc.sync.dma_start(
                                ov[:, h0 + s0:h0 + s0 + 14, :, :],
                                y3t[:, s0:s0 + 14, :, :])
                        pend_out = (ov, h0, y3t)

            # drain the software pipeline; taper the final slices so the
            # post-compute DMA tail is as short as possible
            emit_tail_pair(pend[0], pend[1], pend[2])
            pov, ph0, py3t = pend_out
            for s0, s1 in ((28, 42), (42, 49), (49, 56)):
                nc.sync.dma_start(pov[:, ph0 + s0:ph0 + s1, :, :],
                                  py3t[:, s0:s1, :, :])
    nc.compile()
    return nc


_NC_CACHE = None


def kernel(x, w_head, w_body, w_tail, b_tail):
    global _NC_CACHE, LAST_EXEC_NS
    x = np.asarray(x, dtype=np.float32)
    w_head = np.asarray(w_head, dtype=np.float32)
    w_body = np.asarray(w_body, dtype=np.float32)
    w_tail = np.asarray(w_tail, dtype=np.float32)
    b_tail = np.asarray(b_tail, dtype=np.float32)

    # --- host-side weight prep (tiny) ---
    w_sum = w_head.astype(np.float64).sum(axis=0)          # [COUT, CIN]
    wc = np.einsum("rok,oi->kri", w_body[:, :, :, 0].astype(np.float64),
                   w_sum)                                  # [3, RANK, CIN]
    # wf[p, kh, ko, m] = Wc[kh][m, ko*128+p]
    wf = np.transpose(wc.reshape(3, RANK, KO, 128), (3, 0, 2, 1))
    wf = np.ascontiguousarray(wf.astype(np.float16))

    # wt[p, mo, 0, m]: p<64 -> tap0[r=p]; p>=64 -> tap1
    #   [p, mo, 1, m]: p<64 -> 0;         p>=64 -> tap2
    wt = np.zeros((128, MO, 2, 128), dtype=np.float16)
    wtl = w_tail[:, :, 0, :].reshape(MO, 128, RANK, 3)     # [mo, m, r, kw]
    wt[0:64, :, 0, :] = np.transpose(wtl[:, :, :, 0], (2, 0, 1))
    wt[64:128, :, 0, :] = np.transpose(wtl[:, :, :, 1], (2, 0, 1))
    wt[64:128, :, 1, :] = np.transpose(wtl[:, :, :, 2], (2, 0, 1))

    bias = np.ascontiguousarray(b_tail.reshape(MO, 128).T)  # [128, mo]

    # x -> [128p, B, KO, H, W] fp16 (partition-major for fat DMA lines)
    xp = np.ascontiguousarray(
        x.astype(np.float16).reshape(B, KO, 128, H, W).transpose(2, 0, 1, 3, 4))

    if _NC_CACHE is None:
        _NC_CACHE = _build()
    nc = _NC_CACHE

    in_maps = [
        {"x": np.ascontiguousarray(xp[:, c * BL:(c + 1) * BL]),
         "wf": wf, "wt": wt, "bias": bias}
        for c in range(NCORES)
    ]
    global LAST_IN_MAPS
    LAST_IN_MAPS = in_maps
    trace = os.environ.get("KBENCH_TRACE", "0") == "1"
    res = run_bass_kernel_spmd(nc, in_maps, core_ids=list(range(NCORES)),
                               trace=trace)
    LAST_EXEC_NS = res.exec_time_ns
    # o: [128, BL, H, MO, W] per core -> [B, COUT, H, W]
    o = np.stack([r["o"] for r in res.results], axis=0)    # [NC,128,BL,H,MO,W]
    o = o.transpose(0, 2, 4, 1, 3, 5).reshape(B, COUT, H, W)
    return o.astype(np.float32)


# revision 31
# speedup vs baseline: 1.1675x; 1.1675x over previous
"""CPDBlock (rank-decomposed conv block) Trainium2 kernel.

Reference computation (per image):
  y1 = (sum_r w_head[r]) @ x            # 1x1 conv, 256->256
  y2 = conv_(3,1)(y1, w_body)           # 256->64, pad (1,0) in H
  y3 = conv_(1,3)(y2, w_tail) + b_tail  # 64->256, pad (0,1) in W

Algebraic fusion: head folds into body since both are linear:
  y2[r,h,w] = sum_kh (w_body[:, :, kh] @ w_sum) @ x[:, h+kh-1, w]
so the kernel only runs two conv stages:
  fused:  Wc[kh] = w_body[kh] @ w_head.sum(0)  (3x [64,256], host-side)
  tail:   w_tail as-is, bias applied on the PSUM->SBUF evacuation.

Sharding: data-parallel over batch, 16 images / 8 cores = 2 images/core.
Everything on the PE runs fp16 (x, fused weights, y2, tail weights) with
fp32 PSUM accumulation; rel err ~5e-4.

HBM layouts are partition-major ([128, ...]) so DMA descriptor lines are
3.6KB contiguous instead of 224B; the host transposes x/o accordingly.

Per core, per image, H in chunks of HC rows; groups of NR=4 output rows:
  x chunk  [128p=cin%128, 2=cin//128, HC+2 rows (halo), 112]  (SBUF, f16)
  y2d      [128p, HC rows, 114] f16: partitions 0-63 hold y2 row-padded
           (col0=0, cols1..112=data), partitions 64-127 the same shifted
           one col left (cols0..111=data, col112=0).  The tail's three
           shifted W-windows then come out as two K=128 matmuls.
  y3 stage [128p=cout%128, 2=cout//128, HC, 112] -> DMA out.

Every matmul is M=64 col-tiled (tile_position (0,0)/(0,64)) so the PE
array never changes tiling mode: fused pairs two row-groups across the
col halves; the tail splits its M=128 output across them.  Tail matmuls
for pair p are emitted after the fused matmuls of pair p+1 so the
PSUM->y2d evacuation latency stays off the tensor critical path.
"""
import os

import numpy as np

import concourse.bass as bass
import concourse.mybir as mybir
import concourse.tile as tile
from concourse import bacc
from concourse.bass_utils import run_bass_kernel_spmd

F32 = mybir.dt.float32
F16 = mybir.dt.float16

B, CIN, COUT, RANK, H, W = 16, 256, 256, 64, 112, 112
NCORES = 8
BL = B // NCORES          # images per core
KO = CIN // 128           # input-channel k-tiles
MO = COUT // 128          # output-channel m-tiles
HC = 56                   # rows per chunk
NCH = H // HC             # chunks per image
NR = 4                    # output rows per matmul group (N = NR*112 = 448)
NG = HC // NR             # groups per chunk

LAST_EXEC_NS = None
LAST_IN_MAPS = None


def _build():
    nc = bacc.Bacc("TRN2", target_bir_lowering=False, debug=False,
                   num_devices=NCORES)
    x_d = nc.dram_tensor("x", [128, BL, KO, H, W], F16, kind="ExternalInput")
    wf_d = nc.dram_tensor("wf", [128, 3, KO, RANK], F16, kind="ExternalInput")
    wt_d = nc.dram_tensor("wt", [128, MO, 2, 128], F16, kind="ExternalInput")
    bias_d = nc.dram_tensor("bias", [128, MO], F32, kind="ExternalInput")
    o_d = nc.dram_tensor("o", [128, BL, H, MO, W], F16, kind="ExternalOutput")

    with tile.TileContext(nc) as tc:
        with (
            tc.tile_pool(name="wpool", bufs=1) as wpool,
            tc.tile_pool(name="xpool", bufs=2) as xpool,
            tc.tile_pool(name="ypool", bufs=1) as ypool,
            tc.tile_pool(name="opool", bufs=2) as opool,
            tc.tile_pool(name="psf", bufs=4, space="PSUM") as psf,
            tc.tile_pool(name="pst", bufs=2, space="PSUM") as pst,
        ):
            wf = wpool.tile([128, 3, KO, RANK], F16)
            wt = wpool.tile([128, MO, 2, 128], F16)
            bias = wpool.tile([128, MO], F32)
            # Weights ride the Scalar engine's HWDGE ring so the Sync
            # ring's first issue is chunk 0's x piece (startup latency).
            nc.scalar.dma_start(wf[:], wf_d[:])
            nc.scalar.dma_start(wt[:], wt_d[:])
            nc.scalar.dma_start(bias[:], bias_d[:])

            # Two persistent y2d buffers, alternated per chunk.  Pad
            # columns (col0 of the top half, col112 of the bottom half)
            # are zeroed once and never rewritten.
            y2ds = [ypool.tile([128, HC, 114], F16, tag=f"y2d{i}",
                               name=f"y2d{i}")
                    for i in range(2)]
            # y2d pad zeroing rides GpSimd (idle early) so DVE is free
            # for the first evacuations the moment data arrives.
            for y2d in y2ds:
                nc.gpsimd.memset(y2d[:], 0.0)

            # HAM warm-up: throwaway col-tiled matmuls keep the PE busy
            # from ~5us so the clock gate opens (K=8/8) before the
            # first real matmul; their PSUM bank is overwritten
            # (start=True) later.  They read the wf tile BEFORE its DMA
            # (garbage is fine, results are discarded); the WAR edge
            # delays the wf load to ~10us, still ahead of the first
            # real matmul's x data.
            wv = wf[:].rearrange("p a b c -> p (a b c)")
            wpsum = pst.tile([128, NR * W], F32, tag="pt0", name="pt0")
            for i in range(16):
                ct = 64 * (i % 2)
                nc.tensor.matmul(wpsum[ct:ct + 64, 0:384], wv[:, 0:64],
                                 wv[:], start=True, stop=True,
                                 tile_position=(0, ct))

            it = 0
            pend = None       # deferred tail pair (gp, y2d, y3t)
            pend_out = None   # deferred second output slice (ov, h0, y3t)
            for b in range(BL):
                xv = x_d.ap()[:, b]           # [128, KO, H, W]
                ov = o_d.ap()[:, b]           # [128, H, MO, W]
                for ch in range(NCH):
                    h0 = ch * HC
                    xt = xpool.tile([128, KO, HC + 2, W], F16)
                    # xt slot i holds absolute image row h0 + i - 1;
                    # edge chunks leave the out-of-image slot unwritten
                    # and skip the matmul term that would read it.
                    # Rows h0-1, h0 are copied from the previous chunk's
                    # tile; each image row is DMA'd from HBM once.  The
                    # load is split into 8-row pieces so pair p's
                    # matmuls only wait for piece p.
                    if ch == 0:
                        lo = 1
                    else:
                        nc.gpsimd.tensor_copy(xt[:, :, 0:2, :],
                                              xt_prev[:, :, HC:HC + 2, :])
                        lo = 2
                    hi = HC + 2 if ch < NCH - 1 else HC + 1
                    bounds = [lo] + list(range(10, hi, 8)) + [hi]
                    bounds = sorted(set(b_ for b_ in bounds if lo <= b_ <= hi))
                    for s0, s1 in zip(bounds[:-1], bounds[1:]):
                        nc.sync.dma_start(
                            xt[:, :, s0:s1, :],
                            xv[:, :, h0 + s0 - 1:h0 + s1 - 1, :])
                    xt_prev = xt

                    y2d = y2ds[it % 2]
                    it += 1
                    # [HC, MO, W] free layout so the out-DMA is one
                    # contiguous 12.5KB run per partition per slice.
                    y3t = opool.tile([128, HC, MO, W], F16)

                    # ---- emitters -------------------------------------
                    def emit_fused_pair(gp, xt=xt, ch=ch):
                        """Fused-stage matmuls for groups gp, gp+1: group
                        gp lands in PSUM partitions 0:64 (PE col tiles
                        0/1), gp+1 in 64:128 (col tiles 2/3); the two
                        streams run concurrently."""
                        subs = [0, 1] if gp + 1 < NG else [0]
                        pfp = psf.tile([128, NR * W], F32)
                        for ko in range(KO):
                            for kh in (1, 0, 2):
                                for sub in subs:
                                    g = gp + sub
                                    r0 = g * NR
                                    p0 = 64 * sub
                                    out_ap = pfp[p0:p0 + 64, :]
                                    rhs = xt[:, ko, r0 + kh:r0 + kh + NR, :]
                                    if ch == 0 and g == 0 and kh == 0:
                                        # output row 0 has no row above
                                        out_ap = pfp[p0:p0 + 64, W:NR * W]
                                        rhs = xt[:, ko, 1:NR, :]
                                    elif (ch == NCH - 1 and g == NG - 1
                                          and kh == 2):
                                        # last row has no row below
                                        out_ap = pfp[p0:p0 + 64,
                                                     0:(NR - 1) * W]
                                        rhs = xt[:, ko, r0 + 2:r0 + 1 + NR, :]
                                    nc.tensor.matmul(
                                        out_ap,
                                        wf[:, kh, ko, :],
                                        rhs,
                                        start=(ko == 0 and kh == 1),
                                        stop=(ko == KO - 1 and kh == 2),
                                        tile_position=(0, p0),
                                    )
                        return pfp, subs

                    def emit_evac(gp, pfp, subs, y2d=y2d):
                        """PSUM -> y2d padded/shifted layout; one half on
                        ACT, one on DVE so neither engine is the
                        bottleneck."""
                        for sub in subs:
                            r0 = (gp + sub) * NR
                            pf = pfp[64 * sub:64 * sub + 64, :]
                            nc.scalar.copy(y2d[0:64, r0:r0 + NR, 1:113], pf)
                            nc.vector.tensor_copy(
                                y2d[64:128, r0:r0 + NR, 0:112], pf)

                    def emit_tail_pair(gp, y2d, y3t):
                        """Tail matmuls + biased evacuation for groups
                        gp, gp+1.  Each mo half is col-tiled into two
                        concurrent M=64 streams so the PE stays in
                        128x64 mode; bias rides the evacuation (ACT for
                        mo 0, DVE tensor_scalar for mo 1)."""
                        for sub in ([0, 1] if gp + 1 < NG else [0]):
                            g = gp + sub
                            r0 = g * NR
                            for mo in range(MO):
                                pt = pst.tile([128, NR * W], F32,
                                              tag=f"pt{mo}", name=f"pt{mo}")
                                for tap in range(2):
                                    rhs = y2d[:, r0:r0 + NR, tap:tap + 112]
                                    for ct in (0, 64):
                                        nc.tensor.matmul(
                                            pt[ct:ct + 64, :],
                                            wt[:, mo, tap, ct:ct + 64],
                                            rhs,
                                            start=(tap == 0),
                                            stop=(tap == 1),
                                            tile_position=(0, ct),
                                        )
                                out = y3t[:, r0:r0 + NR, mo, :]
                                if mo == 0:
                                    nc.scalar.add(out, pt[:],
                                                  bias[:, mo, None])
                                else:
                                    nc.vector.tensor_scalar_add(
                                        out, pt[:], bias[:, mo, None])

                    # ---- software-pipelined pair loop -----------------
                    # The tail-pair for pair p is emitted after pair
                    # p+1's fused matmuls (crossing chunk boundaries via
                    # `pend`) so the PSUM->y2d evacuation latency never
                    # heads-of-line-blocks the tensor queue.
                    for gp in range(0, NG, 2):
                        pfp, subs = emit_fused_pair(gp)
                        if pend is not None:
                            emit_tail_pair(pend[0], pend[1], pend[2])
                            pend = None
                            if pend_out is not None:
                                pov, ph0, py3t = pend_out
                                nc.gpsimd.dma_start(
                                    pov[:, ph0 + 28:ph0 + 56, :, :],
                                    py3t[:, 28:56, :, :])
                                pend_out = None
                        emit_evac(gp, pfp, subs)
                        pend = (gp, y2d, y3t)

                    # Output leaves on the GpSimd SWDGE ring so it never
                    # queues behind the x input stream.  The HBM layout
                    # interleaves MO inside rows so each partition is one
                    # 12.5KB contiguous run (128 descriptors per DMA
                    # keeps the Q7 descriptor-generation cost low).
                    # Rows 28:56 need the deferred tail pair; that slice
                    # is emitted right after it, next chunk.
                    last = (b == BL - 1 and ch == NCH - 1)
                    if not last:
                        nc.gpsimd.dma_start(ov[:, h0:h0 + 28, :, :],
                                            y3t[:, 0:28, :, :])
                        pend_out = (ov, h0, y3t)
                    else:
                        # Final chunk: x input is fully landed, so the
                        # Sync HWDGE ring is idle — drain the last
                        # output there in 14-row pieces to cut the
                        # post-compute DMA tail.
                        for s0 in range(0, 28, 14):
                            nc.sync.dma_start(
                                ov[:, h0 + s0:h0 + s0 + 14, :, :],
                                y3t[:, s0:s0 + 14, :, :])
                        pend_out = (ov, h0, y3t)

            # drain the software pipeline; taper the final slices so the
            # post-compute DMA tail is as short as possible
            emit_tail_pair(pend[0], pend[1], pend[2])
            pov, ph0, py3t = pend_out
            for s0, s1 in ((28, 42), (42, 49), (49, 56)):
                nc.sync.dma_start(pov[:, ph0 + s0:ph0 + s1, :, :],
                                  py3t[:, s0:s1, :, :])
    nc.compile()
    return nc


_NC_CACHE = None


def kernel(x, w_head, w_body, w_tail, b_tail):
    global _NC_CACHE, LAST_EXEC_NS
    x = np.asarray(x, dtype=np.float32)
    w_head = np.asarray(w_head, dtype=np.float32)
    w_body = np.asarray(w_body, dtype=np.float32)
    w_tail = np.asarray(w_tail, dtype=np.float32)
    b_tail = np.asarray(b_tail, dtype=np.float32)

    # --- host-side weight prep (tiny) ---
    w_sum = w_head.astype(np.float64).sum(axis=0)          # [COUT, CIN]
    wc = np.einsum("rok,oi->kri", w_body[:, :, :, 0].astype(np.float64),
                   w_sum)                                  # [3, RANK, CIN]
    # wf[p, kh, ko, m] = Wc[kh][m, ko*128+p]
    wf = np.transpose(wc.reshape(3, RANK, KO, 128), (3, 0, 2, 1))
    wf = np.ascontiguousarray(wf.astype(np.float16))

    # wt[p, mo, 0, m]: p<64 -> tap0[r=p]; p>=64 -> tap1
    #   [p, mo, 1, m]: p<64 -> 0;         p>=64 -> tap2
    wt = np.zeros((128, MO, 2, 128), dtype=np.float16)
    wtl = w_tail[:, :, 0, :].reshape(MO, 128, RANK, 3)     # [mo, m, r, kw]
    wt[0:64, :, 0, :] = np.transpose(wtl[:, :, :, 0], (2, 0, 1))
    wt[64:128, :, 0, :] = np.transpose(wtl[:, :, :, 1], (2, 0, 1))
    wt[64:128, :, 1, :] = np.transpose(wtl[:, :, :, 2], (2, 0, 1))

    bias = np.ascontiguousarray(b_tail.reshape(MO, 128).T)  # [128, mo]

    # x -> [128p, B, KO, H, W] fp16 (partition-major for fat DMA lines)
    xp = np.ascontiguousarray(
        x.astype(np.float16).reshape(B, KO, 128, H, W).transpose(2, 0, 1, 3, 4))

    if _NC_CACHE is None:
        _NC_CACHE = _build()
    nc = _NC_CACHE

    in_maps = [
        {"x": np.ascontiguousarray(xp[:, c * BL:(c + 1) * BL]),
         "wf": wf, "wt": wt, "bias": bias}
        for c in range(NCORES)
    ]
    global LAST_IN_MAPS
    LAST_IN_MAPS = in_maps
    trace = os.environ.get("KBENCH_TRACE", "0") == "1"
    res = run_bass_kernel_spmd(nc, in_maps, core_ids=list(range(NCORES)),
                               trace=trace)
    LAST_EXEC_NS = res.exec_time_ns
    # o: [128, BL, H, MO, W] per core -> [B, COUT, H, W]
    o = np.stack([r["o"] for r in res.results], axis=0)    # [NC,128,BL,H,MO,W]
    o = o.transpose(0, 2, 4, 1, 3, 5).reshape(B, COUT, H, W)
    return o.astype(np.float32)


# revision 33
# speedup vs baseline: 1.1855x; 1.0154x over previous
"""CPDBlock (rank-decomposed conv block) Trainium2 kernel.

Reference computation (per image):
  y1 = (sum_r w_head[r]) @ x            # 1x1 conv, 256->256
  y2 = conv_(3,1)(y1, w_body)           # 256->64, pad (1,0) in H
  y3 = conv_(1,3)(y2, w_tail) + b_tail  # 64->256, pad (0,1) in W

Algebraic fusion: head folds into body since both are linear:
  y2[r,h,w] = sum_kh (w_body[:, :, kh] @ w_sum) @ x[:, h+kh-1, w]
so the kernel only runs two conv stages:
  fused:  Wc[kh] = w_body[kh] @ w_head.sum(0)  (3x [64,256], host-side)
  tail:   w_tail as-is, bias applied on the PSUM->SBUF evacuation.

Sharding: data-parallel over batch, 16 images / 8 cores = 2 images/core.
Everything on the PE runs fp16 (x, fused weights, y2, tail weights) with
fp32 PSUM accumulation; rel err ~5e-4.

HBM layouts are partition-major ([128, ...]) so DMA descriptor lines are
3.6KB contiguous instead of 224B; the host transposes x/o accordingly.

Per core, per image, H in chunks of HC rows; groups of NR=4 output rows:
  x chunk  [128p=cin%128, 2=cin//128, HC+2 rows (halo), 112]  (SBUF, f16)
  y2d      [128p, HC rows, 114] f16: partitions 0-63 hold y2 row-padded
           (col0=0, cols1..112=data), partitions 64-127 the same shifted
           one col left (cols0..111=data, col112=0).  The tail's three
           shifted W-windows then come out as two K=128 matmuls.
  y3 stage [128p=cout%128, HC, 2=cout//128, 112] -> DMA out (one
           contiguous run per partition per slice).

Every matmul is M=64 col-tiled (tile_position (0,0)/(0,64)) so the PE
array never changes tiling mode: fused pairs two row-groups across the
col halves; the tail splits its M=128 output across them.  Tail matmuls
for pair p are emitted after the fused matmuls of pair p+1 so the
PSUM->y2d evacuation latency stays off the tensor critical path.
"""
import os

import numpy as np

import concourse.mybir as mybir
import concourse.tile as tile
from concourse import bacc
from concourse.bass_utils import run_bass_kernel_spmd

F32 = mybir.dt.float32
F16 = mybir.dt.float16

B, CIN, COUT, RANK, H, W = 16, 256, 256, 64, 112, 112
NCORES = 8
BL = B // NCORES          # images per core
KO = CIN // 128           # input-channel k-tiles
MO = COUT // 128          # output-channel m-tiles
HC = 56                   # rows per chunk
NCH = H // HC             # chunks per image
NR = 4                    # output rows per matmul group (N = NR*112 = 448)
NG = HC // NR             # groups per chunk

LAST_EXEC_NS = None
LAST_IN_MAPS = None


def _build():
    nc = bacc.Bacc("TRN2", target_bir_lowering=False, debug=False,
                   num_devices=NCORES)
    x_d = nc.dram_tensor("x", [128, BL, KO, H, W], F16, kind="ExternalInput")
    wf_d = nc.dram_tensor("wf", [128, 3, KO, RANK], F16, kind="ExternalInput")
    wt_d = nc.dram_tensor("wt", [128, MO, 2, 128], F16, kind="ExternalInput")
    bias_d = nc.dram_tensor("bias", [128, MO], F32, kind="ExternalInput")
    o_d = nc.dram_tensor("o", [128, BL, H, MO, W], F16, kind="ExternalOutput")

    with tile.TileContext(nc) as tc:
        with (
            tc.tile_pool(name="wpool", bufs=1) as wpool,
            tc.tile_pool(name="xpool", bufs=2) as xpool,
            tc.tile_pool(name="ypool", bufs=1) as ypool,
            tc.tile_pool(name="opool", bufs=2) as opool,
            tc.tile_pool(name="psf", bufs=4, space="PSUM") as psf,
            tc.tile_pool(name="pst", bufs=2, space="PSUM") as pst,
        ):
            wf = wpool.tile([128, 3, KO, RANK], F16)
            wt = wpool.tile([128, MO, 2, 128], F16)
            bias = wpool.tile([128, MO], F32)
            # Weights ride the Scalar engine's HWDGE ring so the Sync
            # ring's first issue is chunk 0's x piece (startup latency).
            nc.scalar.dma_start(wf[:], wf_d[:])
            nc.scalar.dma_start(wt[:], wt_d[:])
            nc.scalar.dma_start(bias[:], bias_d[:])

            # Two persistent y2d buffers, alternated per chunk.  Pad
            # columns (col0 of the top half, col112 of the bottom half)
            # are zeroed once and never rewritten.
            y2ds = [ypool.tile([128, HC, 114], F16, tag=f"y2d{i}",
                               name=f"y2d{i}")
                    for i in range(2)]
            # y2d pad zeroing rides GpSimd (idle early) so DVE is free
            # for the first evacuations the moment data arrives.
            for y2d in y2ds:
                nc.gpsimd.memset(y2d[:], 0.0)

            # HAM warm-up: throwaway col-tiled matmuls keep the PE busy
            # from ~5us so the clock gate opens (K=8/8) before the
            # first real matmul; their PSUM bank is overwritten
            # (start=True) later.  They read the wf tile BEFORE its DMA
            # (garbage is fine, results are discarded); the WAR edge
            # delays the wf load to ~10us, still ahead of the first
            # real matmul's x data.
            wv = wf[:].rearrange("p a b c -> p (a b c)")
            wpsum = pst.tile([128, NR * W], F32, tag="pt0", name="pt0")
            for i in range(16):
                ct = 64 * (i % 2)
                nc.tensor.matmul(wpsum[ct:ct + 64, 0:384], wv[:, 0:64],
                                 wv[:], start=True, stop=True,
                                 tile_position=(0, ct))

            it = 0
            pend = None       # deferred tail pair (gp, y2d, y3t)
            pend_out = None   # deferred second output slice (ov, h0, y3t)
            for b in range(BL):
                xv = x_d.ap()[:, b]           # [128, KO, H, W]
                ov = o_d.ap()[:, b]           # [128, H, MO, W]
                for ch in range(NCH):
                    h0 = ch * HC
                    xt = xpool.tile([128, KO, HC + 2, W], F16)
                    # xt slot i holds absolute image row h0 + i - 1;
                    # edge chunks leave the out-of-image slot unwritten
                    # and skip the matmul term that would read it.
                    # Rows h0-1, h0 are copied from the previous chunk's
                    # tile; each image row is DMA'd from HBM once.  The
                    # load is split into 8-row pieces so pair p's
                    # matmuls only wait for piece p.
                    if ch == 0:
                        lo = 1
                    else:
                        nc.gpsimd.tensor_copy(xt[:, :, 0:2, :],
                                              xt_prev[:, :, HC:HC + 2, :])
                        lo = 2
                    hi = HC + 2 if ch < NCH - 1 else HC + 1
                    bounds = [lo] + list(range(10, hi, 8)) + [hi]
                    bounds = sorted(set(b_ for b_ in bounds if lo <= b_ <= hi))
                    for s0, s1 in zip(bounds[:-1], bounds[1:]):
                        nc.sync.dma_start(
                            xt[:, :, s0:s1, :],
                            xv[:, :, h0 + s0 - 1:h0 + s1 - 1, :])
                    xt_prev = xt

                    y2d = y2ds[it % 2]
                    it += 1
                    # [HC, MO, W] free layout so the out-DMA is one
                    # contiguous 12.5KB run per partition per slice.
                    y3t = opool.tile([128, HC, MO, W], F16)

                    # ---- emitters -------------------------------------
                    def emit_fused_pair(gp, xt=xt, ch=ch):
                        """Fused-stage matmuls for groups gp, gp+1: group
                        gp lands in PSUM partitions 0:64 (PE col tiles
                        0/1), gp+1 in 64:128 (col tiles 2/3); the two
                        streams run concurrently."""
                        subs = [0, 1] if gp + 1 < NG else [0]
                        pfp = psf.tile([128, NR * W], F32)
                        for ko in range(KO):
                            for kh in (1, 0, 2):
                                for sub in subs:
                                    g = gp + sub
                                    r0 = g * NR
                                    p0 = 64 * sub
                                    out_ap = pfp[p0:p0 + 64, :]
                                    rhs = xt[:, ko, r0 + kh:r0 + kh + NR, :]
                                    if ch == 0 and g == 0 and kh == 0:
                                        # output row 0 has no row above
                                        out_ap = pfp[p0:p0 + 64, W:NR * W]
                                        rhs = xt[:, ko, 1:NR, :]
                                    elif (ch == NCH - 1 and g == NG - 1
                                          and kh == 2):
                                        # last row has no row below
                                        out_ap = pfp[p0:p0 + 64,
                                                     0:(NR - 1) * W]
                                        rhs = xt[:, ko, r0 + 2:r0 + 1 + NR, :]
                                    nc.tensor.matmul(
                                        out_ap,
                                        wf[:, kh, ko, :],
                                        rhs,
                                        start=(ko == 0 and kh == 1),
                                        stop=(ko == KO - 1 and kh == 2),
                                        tile_position=(0, p0),
                                    )
                        return pfp, subs

                    def emit_evac(gp, pfp, subs, y2d=y2d):
                        """PSUM -> y2d padded/shifted layout; one half on
                        ACT, one on DVE so neither engine is the
                        bottleneck."""
                        for sub in subs:
                            r0 = (gp + sub) * NR
                            pf = pfp[64 * sub:64 * sub + 64, :]
                            nc.scalar.copy(y2d[0:64, r0:r0 + NR, 1:113], pf)
                            nc.vector.tensor_copy(
                                y2d[64:128, r0:r0 + NR, 0:112], pf)

                    def emit_tail_pair(gp, y2d, y3t):
                        """Tail matmuls + biased evacuation for groups
                        gp, gp+1.  Each mo half is col-tiled into two
                        concurrent M=64 streams so the PE stays in
                        128x64 mode; bias rides the evacuation (ACT for
                        mo 0, DVE tensor_scalar for mo 1)."""
                        for sub in ([0, 1] if gp + 1 < NG else [0]):
                            g = gp + sub
                            r0 = g * NR
                            for mo in range(MO):
                                pt = pst.tile([128, NR * W], F32,
                                              tag=f"pt{mo}", name=f"pt{mo}")
                                for tap in range(2):
                                    rhs = y2d[:, r0:r0 + NR, tap:tap + 112]
                                    for ct in (0, 64):
                                        nc.tensor.matmul(
                                            pt[ct:ct + 64, :],
                                            wt[:, mo, tap, ct:ct + 64],
                                            rhs,
                                            start=(tap == 0),
                                            stop=(tap == 1),
                                            tile_position=(0, ct),
                                        )
                                out = y3t[:, r0:r0 + NR, mo, :]
                                if mo == 0:
                                    nc.scalar.add(out, pt[:],
                                                  bias[:, mo, None])
                                else:
                                    nc.vector.tensor_scalar_add(
                                        out, pt[:], bias[:, mo, None])

                    # ---- software-pipelined pair loop -----------------
                    # The tail-pair for pair p is emitted after pair
                    # p+1's fused matmuls (crossing chunk boundaries via
                    # `pend`) so the PSUM->y2d evacuation latency never
                    # heads-of-line-blocks the tensor queue.
                    for gp in range(0, NG, 2):
                        pfp, subs = emit_fused_pair(gp)
                        if pend is not None:
                            emit_tail_pair(pend[0], pend[1], pend[2])
                            pend = None
                            if pend_out is not None:
                                pov, ph0, py3t = pend_out
                                nc.gpsimd.dma_start(
                                    pov[:, ph0 + 28:ph0 + 56, :, :],
                                    py3t[:, 28:56, :, :])
                                pend_out = None
                        emit_evac(gp, pfp, subs)
                        pend = (gp, y2d, y3t)

                    # Output leaves on the GpSimd SWDGE ring so it never
                    # queues behind the x input stream.  The HBM layout
                    # interleaves MO inside rows so each partition is one
                    # 12.5KB contiguous run (128 descriptors per DMA
                    # keeps the Q7 descriptor-generation cost low).
                    # Rows 28:56 need the deferred tail pair; that slice
                    # is emitted right after it, next chunk.
                    last = (b == BL - 1 and ch == NCH - 1)
                    if not last:
                        nc.gpsimd.dma_start(ov[:, h0:h0 + 28, :, :],
                                            y3t[:, 0:28, :, :])
                        pend_out = (ov, h0, y3t)
                    else:
                        # Final chunk: x input is fully landed, so the
                        # Sync HWDGE ring is idle — drain the last
                        # output there in 14-row pieces to cut the
                        # post-compute DMA tail.
                        for s0 in range(0, 28, 14):
                            nc.sync.dma_start(
                                ov[:, h0 + s0:h0 + s0 + 14, :, :],
                                y3t[:, s0:s0 + 14, :, :])
                        pend_out = (ov, h0, y3t)

            # drain the software pipeline; taper the final slices so the
            # post-compute DMA tail is as short as possible
            emit_tail_pair(pend[0], pend[1], pend[2])
            pov, ph0, py3t = pend_out
            for s0, s1 in ((28, 42), (42, 49), (49, 56)):
                nc.sync.dma_start(pov[:, ph0 + s0:ph0 + s1, :, :],
                                  py3t[:, s0:s1, :, :])
    nc.compile()
    return nc


_NC_CACHE = None


def kernel(x, w_head, w_body, w_tail, b_tail):
    global _NC_CACHE, LAST_EXEC_NS
    x = np.asarray(x, dtype=np.float32)
    w_head = np.asarray(w_head, dtype=np.float32)
    w_body = np.asarray(w_body, dtype=np.float32)
    w_tail = np.asarray(w_tail, dtype=np.float32)
    b_tail = np.asarray(b_tail, dtype=np.float32)

    # --- host-side weight prep (tiny) ---
    w_sum = w_head.astype(np.float64).sum(axis=0)          # [COUT, CIN]
    wc = np.einsum("rok,oi->kri", w_body[:, :, :, 0].astype(np.float64),
                   w_sum)                                  # [3, RANK, CIN]
    # wf[p, kh, ko, m] = Wc[kh][m, ko*128+p]
    wf = np.transpose(wc.reshape(3, RANK, KO, 128), (3, 0, 2, 1))
    wf = np.ascontiguousarray(wf.astype(np.float16))

    # wt[p, mo, 0, m]: p<64 -> tap0[r=p]; p>=64 -> tap1
    #   [p, mo, 1, m]: p<64 -> 0;         p>=64 -> tap2
    wt = np.zeros((128, MO, 2, 128), dtype=np.float16)
    wtl = w_tail[:, :, 0, :].reshape(MO, 128, RANK, 3)     # [mo, m, r, kw]
    wt[0:64, :, 0, :] = np.transpose(wtl[:, :, :, 0], (2, 0, 1))
    wt[64:128, :, 0, :] = np.transpose(wtl[:, :, :, 1], (2, 0, 1))
    wt[64:128, :, 1, :] = np.transpose(wtl[:, :, :, 2], (2, 0, 1))

    bias = np.ascontiguousarray(b_tail.reshape(MO, 128).T)  # [128, mo]

    # x -> [128p, B, KO, H, W] fp16 (partition-major for fat DMA lines)
    xp = np.ascontiguousarray(
        x.astype(np.float16).reshape(B, KO, 128, H, W).transpose(2, 0, 1, 3, 4))

    if _NC_CACHE is None:
        _NC_CACHE = _build()
    nc = _NC_CACHE

    in_maps = [
        {"x": np.ascontiguousarray(xp[:, c * BL:(c + 1) * BL]),
         "wf": wf, "wt": wt, "bias": bias}
        for c in range(NCORES)
    ]
    global LAST_IN_MAPS
    LAST_IN_MAPS = in_maps
    trace = os.environ.get("KBENCH_TRACE", "0") == "1"
    res = run_bass_kernel_spmd(nc, in_maps, core_ids=list(range(NCORES)),
                               trace=trace)
    LAST_EXEC_NS = res.exec_time_ns
    # o: [128, BL, H, MO, W] per core -> [B, COUT, H, W]
    o = np.stack([r["o"] for r in res.results], axis=0)    # [NC,128,BL,H,MO,W]
    o = o.transpose(0, 2, 4, 1, 3, 5).reshape(B, COUT, H, W)
    return o.astype(np.float32)
